# revision 8
# baseline (speedup 1.0000x reference)
"""GPPT (GCN + prompt MoE routing) Trainium2 kernel, 8-core SPMD.

Row-shards the N=8192 nodes across 8 NeuronCores (1024 rows each).
v3: feature matrix resident in SBUF; L0 split into two node-halves so
each half's Y1 block finishes early and its AllGather overlaps the
next compute phase (half1's L0, then L1's first k-tile group).

  L0 half h:  TT_h = feature^T @ adjT_blk[:, h]   (single-pass fp32r)
  h0_h^T = relu(W0^T @ TT_h + b0)                 (fp32r)
  Y1s_h  = h0_h @ (W1*8192)                       (fp32r) -> fp16
  AllGather#h (fp16, 0.5MB/rank)                  overlapped
  L1:    h1^T = relu((Y1s^T @ adjT16) * 2^-26 + b1)   (single-pass fp16,
         k-tiles reordered so half0-gathered data is consumed first)
  scores/experts: hc @ [Wp | WppT | pad]          (fp32r, N=256)

Precision: fp32r matmul rounds both operands to a 12-bit significand
(round-to-nearest; decoded exactly via K=1 outer-product probes and
validated against hardware to 4 digits). Host simulation of this exact
scheme on the real inputs gives 0 routing flips and rel err ~2.6e-4,
with a 1.35e-7 worst-row score margin. The L1 adjacency pass tolerates
a single fp16 pass because h1 is mean-dominated (adj >= 0, Y columns
have nonzero means), shrinking the relative impact of rounding noise.
h0/h1 must be stored at >= fp32r precision: fp16 stores flip 1-2 rows.
"""

import os
import numpy as np

import concourse.bass as bass
import concourse.mybir as mybir
import concourse.tile as tile
from concourse import bacc
from concourse.bass_utils import run_bass_kernel_spmd

N = 8192
IN = 512
H = 512
C = 32
E = 7
NCORES = 8
BLK = N // NCORES          # 1024 nodes per core
HB = BLK // 2              # 512-node half-blocks
KT = N // 128              # 64 contraction k-tiles over nodes
SCALE = 8192.0             # L1 fp16 pre-scale (exact power of two)
NW = E + E * C             # 231 useful expert columns
NWP = 256                  # padded to 256 so fp32r runs 1 cycle/row

F32 = mybir.dt.float32
F32R = mybir.dt.float32r
F16 = mybir.dt.float16

LAST_RESULTS = None
_CACHED_NC = None


def _kernel_body(ctx, tc, aps):
    nc = tc.nc
    AFT = mybir.ActivationFunctionType
    ALU = mybir.AluOpType

    A32a, A32b, A16 = aps["A32a"], aps["A32b"], aps["A16"]
    Fr = aps["Fr"]
    W0r, W1r = aps["W0r"], aps["W1r"]
    b0, b1 = aps["b0"], aps["b1"]
    Wcat = aps["Wcat"]          # [2H, 256] = [Wp | WppT | 0pad]
    iota7 = aps["iota7"]        # [128, 7] fp32 0..6 per row
    out = aps["out"]
    cc_in = aps["cc_in"]
    cc_out = [aps["cc_out0"], aps["cc_out1"]]

    fres = ctx.enter_context(tc.tile_pool(name="fres", bufs=1))
    const = ctx.enter_context(tc.tile_pool(name="const", bufs=1))
    h0pool = ctx.enter_context(tc.tile_pool(name="h0pool", bufs=1))
    scratch = ctx.enter_context(tc.tile_pool(name="scratch", bufs=1))
    stream = ctx.enter_context(tc.tile_pool(name="stream", bufs=3))
    l1s = ctx.enter_context(tc.tile_pool(name="l1s", bufs=2))
    ypool = ctx.enter_context(tc.tile_pool(name="ypool", bufs=2))
    small = ctx.enter_context(tc.tile_pool(name="small", bufs=2))
    psum = ctx.enter_context(tc.tile_pool(name="psum", bufs=1, space="PSUM"))

    ps = [psum.tile([128, 512], F32, name=f"bank{i}") for i in range(8)]

    # ---- L0 half 0: stream A-half0 + load F resident; accumulate ps[0:4]
    ftiles = []
    for k in range(KT):
        ft = fres.tile([128, IN], F32R, name=f"f_{k}")
        at = stream.tile([128, HB], F32R, name="a0")
        r = slice(k * 128, (k + 1) * 128)
        nc.sync.dma_start(ft[:], Fr[r, :])
        nc.sync.dma_start(at[:], A32a[r, :])
        ftiles.append(ft)
        for m in range(4):
            nc.tensor.matmul(
                ps[m][:], ft[:, m * 128:(m + 1) * 128], at[:],
                start=(k == 0), stop=(k == KT - 1),
            )

    # ---- constants / weights (emitted after half0 streams; needed ~L0h0 end)
    w0_t = []
    w1_t = []
    for k in range(4):
        t = const.tile([128, H], F32R, name=f"w0_{k}")
        nc.sync.dma_start(t[:], W0r[k * 128:(k + 1) * 128, :])
        w0_t.append(t)
        t = const.tile([128, H], F32R, name=f"w1_{k}")
        nc.sync.dma_start(t[:], W1r[k * 128:(k + 1) * 128, :])
        w1_t.append(t)
    b0_t = []
    b1_t = []
    for m in range(4):
        t = const.tile([128, 1], F32, name=f"b0_{m}")
        nc.sync.dma_start(t[:], b0[m * 128:(m + 1) * 128, :])
        b0_t.append(t)
        t = const.tile([128, 1], F32, name=f"b1_{m}")
        nc.sync.dma_start(t[:], b1[m * 128:(m + 1) * 128, :])
        b1_t.append(t)

    h0t = [h0pool.tile([128, BLK], F32R, name=f"h0t_{m}") for m in range(4)]

    def half_tail(h, tt_ps, h0_ps, y_ps):
        """tt copy -> h0 -> Y1 -> cc_in for node-half h of this block."""
        cs = slice(h * HB, (h + 1) * HB)
        tt = []
        for m in range(4):
            t = scratch.tile([128, HB], F32R, name=f"s_{h * 4 + m}")
            nc.vector.tensor_copy(t[:], ps[tt_ps + m][:])
            tt.append(t)
        for m in range(4):
            pt = ps[h0_ps + m]
            for k in range(4):
                nc.tensor.matmul(
                    pt[:], w0_t[k][:, m * 128:(m + 1) * 128], tt[k][:],
                    start=(k == 0), stop=(k == 3),
                )
            nc.scalar.activation(
                h0t[m][:, cs], pt[:], AFT.Relu, bias=b0_t[m][:], scale=1.0,
            )
        for mc in range(4):
            pt = ps[y_ps + mc]
            col = h * HB + mc * 128
            for k in range(4):
                nc.tensor.matmul(
                    pt[:], h0t[k][:, col:col + 128], w1_t[k][:],
                    start=(k == 0), stop=(k == 3),
                )
            yh = ypool.tile([128, H], F16, name="yh")
            nc.vector.tensor_copy(yh[:], pt[:])
            nc.sync.dma_start(cc_in[h * HB + mc * 128:h * HB + (mc + 1) * 128, :],
                              yh[:])
        nc.gpsimd.collective_compute(
            "AllGather",
            mybir.AluOpType.bypass,
            replica_groups=[list(range(NCORES))],
            ins=[cc_in[cs, :].opt()],
            outs=[cc_out[h].opt()],
        )

    half_tail(0, tt_ps=0, h0_ps=4, y_ps=0)

    # ---- L0 half 1: F resident, stream A-half1; accumulate ps[4:8]
    for k in range(KT):
        at = stream.tile([128, HB], F32R, name="a1")
        r = slice(k * 128, (k + 1) * 128)
        nc.sync.dma_start(at[:], A32b[r, :])
        for m in range(4):
            nc.tensor.matmul(
                ps[4 + m][:], ftiles[k][:, m * 128:(m + 1) * 128], at[:],
                start=(k == 0), stop=(k == KT - 1),
            )

    # expert weights: needed last, emit DMA late
    wcat_t = []
    for k in range(8):
        t = const.tile([128, NWP], F32R, name=f"wcat_{k}")
        nc.sync.dma_start(t[:], Wcat[k * 128:(k + 1) * 128, :])
        wcat_t.append(t)
    iota_t = const.tile([128, E], F32, name="iota7")
    nc.sync.dma_start(iota_t[:], iota7[:, :])

    half_tail(1, tt_ps=4, h0_ps=0, y_ps=4)

    # =========== L1: h1T[m,n] = sum_k Y[k][:,m].T @ A16[k][:,n] (fp16) ===
    # k-tiles reordered: all half0-sourced tiles first (cc_out0), then half1.
    korder = [kk for kk in range(KT) if kk % 8 < 4] + \
             [kk for kk in range(KT) if kk % 8 >= 4]
    for pos, kk in enumerate(korder):
        g = kk * 128
        rank, w = g // BLK, g % BLK
        src = cc_out[0] if w < HB else cc_out[1]
        row = rank * HB + (w % HB)
        yk = l1s.tile([128, H], F16, name="yk")
        ah = l1s.tile([128, BLK], F16, name="ah1")
        nc.sync.dma_start(yk[:], src[row:row + 128, :])
        nc.sync.dma_start(ah[:], A16[kk * 128:(kk + 1) * 128, :])
        for m in range(4):
            for n in range(2):
                nc.tensor.matmul(
                    ps[m * 2 + n][:],
                    yk[:, m * 128:(m + 1) * 128],
                    ah[:, n * 512:(n + 1) * 512],
                    start=(pos == 0),
                    stop=(pos == KT - 1),
                )

    h1t = []
    for m in range(4):
        for n in range(2):
            t = scratch.tile([128, HB], F32R, name=f"s_{m * 2 + n}")
            nc.scalar.activation(
                t[:], ps[m * 2 + n][:],
                AFT.Relu, bias=b1_t[m][:], scale=1.0 / (SCALE * SCALE),
            )
            h1t.append(t)

    # =========== scores + all-expert heads + one-hot select ==============
    # hc^T k-chunks 0..7: relu(h1) halves [m,n]; 8..: h0 tiles
    for mc in range(8):
        pt = ps[mc]
        # h1 part: hc dims 0:512 -> tiles h1t[k*2 + nh] hold
        # [128 h1 dims, 512 nodes of half nh]
        nh = mc // 4          # node half of this m-chunk
        off = (mc % 4) * 128  # node offset within half
        for k in range(4):
            nc.tensor.matmul(
                pt[:, 0:NWP],
                h1t[k * 2 + nh][:, off:off + 128],
                wcat_t[k][:],
                start=(k == 0), stop=False,
            )
        for k in range(4):
            nc.tensor.matmul(
                pt[:, 0:NWP],
                h0t[k][:, mc * 128:(mc + 1) * 128],
                wcat_t[4 + k][:],
                start=False, stop=(k == 3),
            )
        sc = pt[:, 0:E]
        oa = pt[:, E:NW]
        rmax = small.tile([128, 1], F32, name="rmax")
        nc.vector.tensor_reduce(rmax[:], sc, axis=mybir.AxisListType.X, op=ALU.max)
        val = small.tile([128, E], F32, name="val")
        nc.vector.tensor_scalar(val[:], sc, rmax[:], 1024.0, ALU.is_lt, ALU.mult)
        nc.vector.tensor_tensor(val[:], val[:], iota_t[:], op=ALU.add)
        idxf = small.tile([128, 1], F32, name="idxf")
        nc.vector.tensor_reduce(idxf[:], val[:], axis=mybir.AxisListType.X, op=ALU.min)
        onehot = small.tile([128, E], F32, name="onehot")
        nc.vector.tensor_scalar(onehot[:], val[:], idxf[:], None, ALU.is_equal)
        masked = small.tile([128, E, C], F32, name="masked")
        oa_v = oa.rearrange("p (e c) -> p e c", e=E)
        oh_v = onehot[:, :, None].broadcast_to((128, E, C))
        nc.vector.tensor_tensor(masked[:], oa_v, oh_v, op=ALU.mult)
        out_m = small.tile([128, C], F32, name="out_m")
        mv = masked[:].rearrange("p e c -> p c e")
        nc.vector.tensor_reduce(out_m[:], mv, axis=mybir.AxisListType.X, op=ALU.add)
        nc.sync.dma_start(out[mc * 128:(mc + 1) * 128, :], out_m[:])


def _build_nc():
    nc = bacc.Bacc("TRN2", target_bir_lowering=False, debug=False,
                   num_devices=NCORES)
    aps = {}
    def inp(name, shape, dt):
        aps[name] = nc.dram_tensor(name, shape, dt, kind="ExternalInput").ap()
    inp("A32a", [N, HB], F32R)
    inp("A32b", [N, HB], F32R)
    inp("A16", [N, BLK], F16)
    inp("Fr", [N, IN], F32R)
    inp("W0r", [IN, H], F32R)
    inp("W1r", [H, H], F32R)
    inp("b0", [H, 1], F32)
    inp("b1", [H, 1], F32)
    inp("Wcat", [2 * H, NWP], F32R)
    inp("iota7", [128, E], F32)
    aps["out"] = nc.dram_tensor("out", [BLK, C], F32, kind="ExternalOutput").ap()
    aps["cc_in"] = nc.dram_tensor("cc_in", [BLK, H], F16).ap()
    aps["cc_out0"] = nc.dram_tensor("cc_out0", [NCORES * HB, H], F16,
                                    addr_space="Shared").ap()
    aps["cc_out1"] = nc.dram_tensor("cc_out1", [NCORES * HB, H], F16,
                                    addr_space="Shared").ap()
    from contextlib import ExitStack
    with tile.TileContext(nc) as tc, ExitStack() as ctx:
        _kernel_body(ctx, tc, aps)
    nc.compile()
    return nc


def kernel(feature, adj, W0, b0, W1, b1, Wp, Wpp):
    global LAST_RESULTS, _CACHED_NC
    feature = np.ascontiguousarray(np.asarray(feature, dtype=np.float32))
    adj = np.asarray(adj, dtype=np.float32)
    W0 = np.asarray(W0, dtype=np.float32)
    b0 = np.asarray(b0, dtype=np.float32)
    W1 = np.asarray(W1, dtype=np.float32)
    b1 = np.asarray(b1, dtype=np.float32)
    Wp = np.asarray(Wp, dtype=np.float32)
    Wpp = np.asarray(Wpp, dtype=np.float32)

    if _CACHED_NC is None:
        _CACHED_NC = _build_nc()
    nc = _CACHED_NC

    Wcat = np.concatenate(
        [Wp, Wpp.transpose(1, 0, 2).reshape(2 * H, E * C),
         np.zeros((2 * H, NWP - NW), np.float32)], axis=1)
    Wcat = np.ascontiguousarray(Wcat)
    iota7 = np.tile(np.arange(E, dtype=np.float32), (128, 1))
    shared = {
        "Fr": feature,
        "W0r": np.ascontiguousarray(W0),
        "W1r": np.ascontiguousarray(W1 * SCALE),
        "b0": b0.reshape(H, 1), "b1": b1.reshape(H, 1),
        "Wcat": Wcat, "iota7": iota7,
    }
    in_maps = []
    for c in range(NCORES):
        blk = np.ascontiguousarray(adj[c * BLK:(c + 1) * BLK, :].T)
        m = dict(shared)
        m["A32a"] = np.ascontiguousarray(blk[:, 0:HB])
        m["A32b"] = np.ascontiguousarray(blk[:, HB:BLK])
        m["A16"] = np.ascontiguousarray((blk * SCALE).astype(np.float16))
        in_maps.append(m)

    trace = os.environ.get("BASS_KERNEL_TRACE", "0") == "1"
    res = run_bass_kernel_spmd(nc, in_maps, list(range(NCORES)), trace=trace)
    LAST_RESULTS = res
    out = np.concatenate([res.results[c]["out"] for c in range(NCORES)], axis=0)
    return out


# revision 9
# speedup vs baseline: 1.2813x; 1.2813x over previous
"""GPPT (GCN + prompt MoE routing) Trainium2 kernel, 8-core SPMD.

Row-shards the N=8192 nodes across 8 NeuronCores (1024 rows each).
v4: v2 structure (single AllGather) + late const loads (fast start) +
single-descriptor adj DMAs + local-block L1 overlap: each core's
streamed A16 has its own block's rows zeroed on host, and the local
contribution adj[:, own] @ Y_own runs from SBUF-resident Y tiles right
after the AllGather is issued, so the PE has work during the collective.

  L0:    TT = feature^T @ adjT_blk            (single-pass fp32r)
  h0^T   = relu(W0^T @ TT + b0)               (fp32r)
  Y1s    = h0_blk @ (W1*8192)                 (fp32r) -> fp16
  AllGather(Y1 fp16, 1MB/rank)                local L1 part overlaps
  L1:    h1^T = relu((Y1s^T @ adjT16) * 2^-26 + b1)   (single-pass fp16)
  scores/experts: hc @ [Wp | WppT | pad]      (fp32r, N=256)

Precision: fp32r matmul rounds both operands to a 12-bit significand
(round-to-nearest; decoded exactly via K=1 outer-product probes and
validated against hardware to 4 digits). Host simulation of this exact
scheme on the real inputs gives 0 routing flips and rel err ~2.6e-4,
with a 1.35e-7 worst-row score margin. The L1 adjacency pass tolerates
a single fp16 pass because h1 is mean-dominated (adj >= 0, Y columns
have nonzero means), shrinking the relative impact of rounding noise.
h0/h1 must be stored at >= fp32r precision: fp16 stores flip 1-2 rows.
"""

import os
import numpy as np

import concourse.bass as bass
import concourse.mybir as mybir
import concourse.tile as tile
from concourse import bacc
from concourse.bass_utils import run_bass_kernel_spmd

N = 8192
IN = 512
H = 512
C = 32
E = 7
NCORES = 8
BLK = N // NCORES          # 1024 nodes per core
KT = N // 128              # 64 contraction k-tiles over nodes
KB = BLK // 128            # 8 k-tiles within the local block
SCALE = 8192.0             # L1 fp16 pre-scale (exact power of two)
NW = E + E * C             # 231 useful expert columns
NWP = 256                  # padded to 256 so fp32r runs 1 cycle/row

F32 = mybir.dt.float32
F32R = mybir.dt.float32r
F16 = mybir.dt.float16

LAST_RESULTS = None
_CACHED_NC = None


def _kernel_body(ctx, tc, aps):
    nc = tc.nc
    AFT = mybir.ActivationFunctionType
    ALU = mybir.AluOpType

    A32, A16, A16own = aps["A32"], aps["A16"], aps["A16own"]
    Fr = aps["Fr"]
    W0r, W1r = aps["W0r"], aps["W1r"]
    b0, b1 = aps["b0"], aps["b1"]
    Wcat = aps["Wcat"]          # [2H, 256] = [Wp | WppT | 0pad]
    iota7 = aps["iota7"]        # [128, 7] fp32 0..6 per row
    out = aps["out"]
    cc_in, cc_out = aps["cc_in"], aps["cc_out"]

    const = ctx.enter_context(tc.tile_pool(name="const", bufs=1))
    acts = ctx.enter_context(tc.tile_pool(name="acts", bufs=1))
    stream = ctx.enter_context(tc.tile_pool(name="stream", bufs=4))
    l1s = ctx.enter_context(tc.tile_pool(name="l1s", bufs=4))
    ypool = ctx.enter_context(tc.tile_pool(name="ypool", bufs=1))
    small = ctx.enter_context(tc.tile_pool(name="small", bufs=4))
    psum = ctx.enter_context(tc.tile_pool(name="psum", bufs=1, space="PSUM"))

    ps = [psum.tile([128, 512], F32, name=f"bank{i}") for i in range(8)]

    # =========== L0: TT[m,n] = sum_k F[k][:,m].T @ A[k][:,n] (fp32r) =====
    # const loads are emitted AFTER the streaming loop so the first k-tiles
    # hit the DMA queues immediately at kernel start.
    for k in range(KT):
        ft = stream.tile([128, IN], F32R, name="ft")
        at = stream.tile([128, BLK], F32R, name="at")
        r = slice(k * 128, (k + 1) * 128)
        nc.sync.dma_start(ft[:], Fr[r, :])
        nc.sync.dma_start(at[:], A32[r, :])
        for m in range(4):
            for n in range(2):
                nc.tensor.matmul(
                    ps[m * 2 + n][:],
                    ft[:, m * 128:(m + 1) * 128],
                    at[:, n * 512:(n + 1) * 512],
                    start=(k == 0),
                    stop=(k == KT - 1),
                )

    # ---- weights needed from the h0 phase onward ----
    w0_t = []
    w1_t = []
    for k in range(4):
        t = const.tile([128, H], F32R, name=f"w0_{k}")
        nc.sync.dma_start(t[:], W0r[k * 128:(k + 1) * 128, :])
        w0_t.append(t)
        t = const.tile([128, H], F32R, name=f"w1_{k}")
        nc.sync.dma_start(t[:], W1r[k * 128:(k + 1) * 128, :])
        w1_t.append(t)
    b0_t = []
    b1_t = []
    for m in range(4):
        t = const.tile([128, 1], F32, name=f"b0_{m}")
        nc.sync.dma_start(t[:], b0[m * 128:(m + 1) * 128, :])
        b0_t.append(t)
        t = const.tile([128, 1], F32, name=f"b1_{m}")
        nc.sync.dma_start(t[:], b1[m * 128:(m + 1) * 128, :])
        b1_t.append(t)

    # copy TT out of PSUM
    tt = []
    for m in range(4):
        t = acts.tile([128, BLK], F32R, name=f"tt_{m}")
        for n in range(2):
            nc.vector.tensor_copy(t[:, n * 512:(n + 1) * 512], ps[m * 2 + n][:])
        tt.append(t)

    # =========== h0T[m,n] = relu(sum_k W0[k][:,m].T @ TT[k][:,n] + b0) ===
    h0t = [acts.tile([128, BLK], F32R, name=f"h0t_{m}") for m in range(4)]
    for m in range(4):
        for n in range(2):
            pt = ps[m * 2 + n]
            for k in range(4):
                nc.tensor.matmul(
                    pt[:],
                    w0_t[k][:, m * 128:(m + 1) * 128],
                    tt[k][:, n * 512:(n + 1) * 512],
                    start=(k == 0),
                    stop=(k == 3),
                )
            nc.scalar.activation(
                h0t[m][:, n * 512:(n + 1) * 512], pt[:],
                AFT.Relu, bias=b0_t[m][:], scale=1.0,
            )

    # =========== Y1s[m] = sum_k h0t[k][:,m].T @ W1r[k]  -> fp16 chunks ===
    yloc = []
    for m in range(8):
        pt = ps[m]
        for k in range(4):
            nc.tensor.matmul(
                pt[:],
                h0t[k][:, m * 128:(m + 1) * 128],
                w1_t[k][:],
                start=(k == 0),
                stop=(k == 3),
            )
        yh = ypool.tile([128, H], F16, name=f"yh_{m}")
        nc.vector.tensor_copy(yh[:], pt[:])
        nc.sync.dma_start(cc_in[m * 128:(m + 1) * 128, :], yh[:])
        yloc.append(yh)

    # =========== AllGather Y1 (fp16) across the 8 cores ==================
    nc.gpsimd.collective_compute(
        "AllGather",
        mybir.AluOpType.bypass,
        replica_groups=[list(range(NCORES))],
        ins=[cc_in.opt()],
        outs=[cc_out.opt()],
    )

    # =========== L1 local part: own-block columns from SBUF-resident Y ===
    # A16 (streamed below) has this core's own rows zeroed on host; the own
    # contribution adj[:, own] @ Y_own runs here, overlapping the AllGather.
    for k2 in range(KB):
        ao = l1s.tile([128, BLK], F16, name="ao")
        nc.sync.dma_start(ao[:], A16own[k2 * 128:(k2 + 1) * 128, :])
        for m in range(4):
            for n in range(2):
                nc.tensor.matmul(
                    ps[m * 2 + n][:],
                    yloc[k2][:, m * 128:(m + 1) * 128],
                    ao[:, n * 512:(n + 1) * 512],
                    start=(k2 == 0),
                    stop=False,
                )

    # expert weights: needed last, emit DMA late
    wcat_t = []
    for k in range(8):
        t = const.tile([128, NWP], F32R, name=f"wcat_{k}")
        nc.sync.dma_start(t[:], Wcat[k * 128:(k + 1) * 128, :])
        wcat_t.append(t)
    iota_t = const.tile([128, E], F32, name="iota7")
    nc.sync.dma_start(iota_t[:], iota7[:, :])

    # =========== L1 streamed: all 64 k-tiles (own rows are zeros) ========
    for k in range(KT):
        yk = l1s.tile([128, H], F16, name="yk")
        ah = l1s.tile([128, BLK], F16, name="ah1")
        r = slice(k * 128, (k + 1) * 128)
        nc.sync.dma_start(yk[:], cc_out[r, :])
        nc.sync.dma_start(ah[:], A16[r, :])
        for m in range(4):
            for n in range(2):
                nc.tensor.matmul(
                    ps[m * 2 + n][:],
                    yk[:, m * 128:(m + 1) * 128],
                    ah[:, n * 512:(n + 1) * 512],
                    start=False,
                    stop=(k == KT - 1),
                )

    h1t = [acts.tile([128, BLK], F32R, name=f"h1t_{m}") for m in range(4)]
    for m in range(4):
        for n in range(2):
            nc.scalar.activation(
                h1t[m][:, n * 512:(n + 1) * 512], ps[m * 2 + n][:],
                AFT.Relu, bias=b1_t[m][:], scale=1.0 / (SCALE * SCALE),
            )

    # =========== scores + all-expert heads + one-hot select ==============
    hct = h1t + h0t
    for mc in range(8):
        pt = ps[mc]
        for k in range(8):
            nc.tensor.matmul(
                pt[:, 0:NWP],
                hct[k][:, mc * 128:(mc + 1) * 128],
                wcat_t[k][:],
                start=(k == 0),
                stop=(k == 7),
            )
        sc = pt[:, 0:E]
        oa = pt[:, E:NW]
        rmax = small.tile([128, 1], F32, name="rmax")
        nc.vector.tensor_reduce(rmax[:], sc, axis=mybir.AxisListType.X, op=ALU.max)
        val = small.tile([128, E], F32, name="val")
        nc.vector.tensor_scalar(val[:], sc, rmax[:], 1024.0, ALU.is_lt, ALU.mult)
        nc.vector.tensor_tensor(val[:], val[:], iota_t[:], op=ALU.add)
        idxf = small.tile([128, 1], F32, name="idxf")
        nc.vector.tensor_reduce(idxf[:], val[:], axis=mybir.AxisListType.X, op=ALU.min)
        onehot = small.tile([128, E], F32, name="onehot")
        nc.vector.tensor_scalar(onehot[:], val[:], idxf[:], None, ALU.is_equal)
        masked = small.tile([128, E, C], F32, name="masked")
        oa_v = oa.rearrange("p (e c) -> p e c", e=E)
        oh_v = onehot[:, :, None].broadcast_to((128, E, C))
        nc.vector.tensor_tensor(masked[:], oa_v, oh_v, op=ALU.mult)
        out_m = small.tile([128, C], F32, name="out_m")
        mv = masked[:].rearrange("p e c -> p c e")
        nc.vector.tensor_reduce(out_m[:], mv, axis=mybir.AxisListType.X, op=ALU.add)
        nc.sync.dma_start(out[mc * 128:(mc + 1) * 128, :], out_m[:])


def _build_nc():
    nc = bacc.Bacc("TRN2", target_bir_lowering=False, debug=False,
                   num_devices=NCORES)
    aps = {}
    def inp(name, shape, dt):
        aps[name] = nc.dram_tensor(name, shape, dt, kind="ExternalInput").ap()
    inp("A32", [N, BLK], F32R)
    inp("A16", [N, BLK], F16)
    inp("A16own", [BLK, BLK], F16)
    inp("Fr", [N, IN], F32R)
    inp("W0r", [IN, H], F32R)
    inp("W1r", [H, H], F32R)
    inp("b0", [H, 1], F32)
    inp("b1", [H, 1], F32)
    inp("Wcat", [2 * H, NWP], F32R)
    inp("iota7", [128, E], F32)
    aps["out"] = nc.dram_tensor("out", [BLK, C], F32, kind="ExternalOutput").ap()
    aps["cc_in"] = nc.dram_tensor("cc_in", [BLK, H], F16).ap()
    aps["cc_out"] = nc.dram_tensor("cc_out", [N, H], F16,
                                   addr_space="Shared").ap()
    from contextlib import ExitStack
    with tile.TileContext(nc) as tc, ExitStack() as ctx:
        _kernel_body(ctx, tc, aps)
    nc.compile()
    return nc


def kernel(feature, adj, W0, b0, W1, b1, Wp, Wpp):
    global LAST_RESULTS, _CACHED_NC
    feature = np.ascontiguousarray(np.asarray(feature, dtype=np.float32))
    adj = np.asarray(adj, dtype=np.float32)
    W0 = np.asarray(W0, dtype=np.float32)
    b0 = np.asarray(b0, dtype=np.float32)
    W1 = np.asarray(W1, dtype=np.float32)
    b1 = np.asarray(b1, dtype=np.float32)
    Wp = np.asarray(Wp, dtype=np.float32)
    Wpp = np.asarray(Wpp, dtype=np.float32)

    if _CACHED_NC is None:
        _CACHED_NC = _build_nc()
    nc = _CACHED_NC

    Wcat = np.concatenate(
        [Wp, Wpp.transpose(1, 0, 2).reshape(2 * H, E * C),
         np.zeros((2 * H, NWP - NW), np.float32)], axis=1)
    Wcat = np.ascontiguousarray(Wcat)
    iota7 = np.tile(np.arange(E, dtype=np.float32), (128, 1))
    shared = {
        "Fr": feature,
        "W0r": np.ascontiguousarray(W0),
        "W1r": np.ascontiguousarray(W1 * SCALE),
        "b0": b0.reshape(H, 1), "b1": b1.reshape(H, 1),
        "Wcat": Wcat, "iota7": iota7,
    }
    in_maps = []
    for c in range(NCORES):
        blk = np.ascontiguousarray(adj[c * BLK:(c + 1) * BLK, :].T)
        a16 = (blk * SCALE).astype(np.float16)
        m = dict(shared)
        m["A32"] = blk
        m["A16own"] = np.ascontiguousarray(a16[c * BLK:(c + 1) * BLK, :])
        a16 = a16.copy()
        a16[c * BLK:(c + 1) * BLK, :] = 0
        m["A16"] = np.ascontiguousarray(a16)
        in_maps.append(m)

    trace = os.environ.get("BASS_KERNEL_TRACE", "0") == "1"
    res = run_bass_kernel_spmd(nc, in_maps, list(range(NCORES)), trace=trace)
    LAST_RESULTS = res
    out = np.concatenate([res.results[c]["out"] for c in range(NCORES)], axis=0)
    return out


# revision 10
# speedup vs baseline: 1.2854x; 1.0032x over previous
"""GPPT (GCN + prompt MoE routing) Trainium2 kernel, 8-core SPMD.

Row-shards the N=8192 nodes across 8 NeuronCores (1024 rows each).
v4: v2 structure (single AllGather) + late const loads (fast start) +
single-descriptor adj DMAs + local-block L1 overlap: each core's
streamed A16 has its own block's rows zeroed on host, and the local
contribution adj[:, own] @ Y_own runs from SBUF-resident Y tiles right
after the AllGather is issued, so the PE has work during the collective.

  L0:    TT = feature^T @ adjT_blk            (single-pass fp32r)
  h0^T   = relu(W0^T @ TT + b0)               (fp32r)
  Y1s    = h0_blk @ (W1*8192)                 (fp32r) -> fp16
  AllGather(Y1 fp16, 1MB/rank)                local L1 part overlaps
  L1:    h1^T = relu((Y1s^T @ adjT16) * 2^-26 + b1)   (single-pass fp16)
  scores/experts: hc @ [Wp | WppT | pad]      (fp32r, N=256)

Precision: fp32r matmul rounds both operands to a 12-bit significand
(round-to-nearest; decoded exactly via K=1 outer-product probes and
validated against hardware to 4 digits). Host simulation of this exact
scheme on the real inputs gives 0 routing flips and rel err ~2.6e-4,
with a 1.35e-7 worst-row score margin. The L1 adjacency pass tolerates
a single fp16 pass because h1 is mean-dominated (adj >= 0, Y columns
have nonzero means), shrinking the relative impact of rounding noise.
h0/h1 must be stored at >= fp32r precision: fp16 stores flip 1-2 rows.
"""

import os
import numpy as np

import concourse.bass as bass
import concourse.mybir as mybir
import concourse.tile as tile
from concourse import bacc
from concourse.bass_utils import run_bass_kernel_spmd

N = 8192
IN = 512
H = 512
C = 32
E = 7
NCORES = 8
BLK = N // NCORES          # 1024 nodes per core
KT = N // 128              # 64 contraction k-tiles over nodes
KB = BLK // 128            # 8 k-tiles within the local block
SCALE = 8192.0             # L1 fp16 pre-scale (exact power of two)
NW = E + E * C             # 231 useful expert columns
NWP = 256                  # padded to 256 so fp32r runs 1 cycle/row

F32 = mybir.dt.float32
F32R = mybir.dt.float32r
F16 = mybir.dt.float16

LAST_RESULTS = None
_CACHED_NC = None


def _kernel_body(ctx, tc, aps):
    nc = tc.nc
    AFT = mybir.ActivationFunctionType
    ALU = mybir.AluOpType

    A32, A16, A16own = aps["A32"], aps["A16"], aps["A16own"]
    Fr = aps["Fr"]
    W0r, W1r = aps["W0r"], aps["W1r"]
    b0, b1 = aps["b0"], aps["b1"]
    Wcat = aps["Wcat"]          # [2H, 256] = [Wp | WppT | 0pad]
    iota7 = aps["iota7"]        # [128, 7] fp32 0..6 per row
    out = aps["out"]
    cc_in, cc_out = aps["cc_in"], aps["cc_out"]

    const = ctx.enter_context(tc.tile_pool(name="const", bufs=1))
    acts = ctx.enter_context(tc.tile_pool(name="acts", bufs=1))
    stream = ctx.enter_context(tc.tile_pool(name="stream", bufs=6))
    l1s = ctx.enter_context(tc.tile_pool(name="l1s", bufs=4))
    ypool = ctx.enter_context(tc.tile_pool(name="ypool", bufs=1))
    small = ctx.enter_context(tc.tile_pool(name="small", bufs=4))
    psum = ctx.enter_context(tc.tile_pool(name="psum", bufs=1, space="PSUM"))

    ps = [psum.tile([128, 512], F32, name=f"bank{i}") for i in range(8)]

    # =========== L0: TT[m,n] = sum_k F[k][:,m].T @ A[k][:,n] (fp32r) =====
    # const loads are emitted AFTER the streaming loop so the first k-tiles
    # hit the DMA queues immediately at kernel start.
    for k in range(KT):
        ft = stream.tile([128, IN], F32R, name="ft")
        at = stream.tile([128, BLK], F32R, name="at")
        r = slice(k * 128, (k + 1) * 128)
        nc.sync.dma_start(ft[:], Fr[r, :])
        nc.sync.dma_start(at[:], A32[r, :])
        for m in range(4):
            for n in range(2):
                nc.tensor.matmul(
                    ps[m * 2 + n][:],
                    ft[:, m * 128:(m + 1) * 128],
                    at[:, n * 512:(n + 1) * 512],
                    start=(k == 0),
                    stop=(k == KT - 1),
                )

    # ---- weights needed from the h0 phase onward ----
    w0_t = []
    w1_t = []
    for k in range(4):
        t = const.tile([128, H], F32R, name=f"w0_{k}")
        nc.sync.dma_start(t[:], W0r[k * 128:(k + 1) * 128, :])
        w0_t.append(t)
        t = const.tile([128, H], F32R, name=f"w1_{k}")
        nc.sync.dma_start(t[:], W1r[k * 128:(k + 1) * 128, :])
        w1_t.append(t)
    b0_t = []
    b1_t = []
    for m in range(4):
        t = const.tile([128, 1], F32, name=f"b0_{m}")
        nc.sync.dma_start(t[:], b0[m * 128:(m + 1) * 128, :])
        b0_t.append(t)
        t = const.tile([128, 1], F32, name=f"b1_{m}")
        nc.sync.dma_start(t[:], b1[m * 128:(m + 1) * 128, :])
        b1_t.append(t)

    # copy TT out of PSUM
    tt = []
    for m in range(4):
        t = acts.tile([128, BLK], F32R, name=f"tt_{m}")
        for n in range(2):
            nc.vector.tensor_copy(t[:, n * 512:(n + 1) * 512], ps[m * 2 + n][:])
        tt.append(t)

    # =========== h0T[m,n] = relu(sum_k W0[k][:,m].T @ TT[k][:,n] + b0) ===
    h0t = [acts.tile([128, BLK], F32R, name=f"h0t_{m}") for m in range(4)]
    for m in range(4):
        for n in range(2):
            pt = ps[m * 2 + n]
            for k in range(4):
                nc.tensor.matmul(
                    pt[:],
                    w0_t[k][:, m * 128:(m + 1) * 128],
                    tt[k][:, n * 512:(n + 1) * 512],
                    start=(k == 0),
                    stop=(k == 3),
                )
            nc.scalar.activation(
                h0t[m][:, n * 512:(n + 1) * 512], pt[:],
                AFT.Relu, bias=b0_t[m][:], scale=1.0,
            )

    # =========== Y1s[m] = sum_k h0t[k][:,m].T @ W1r[k]  -> fp16 chunks ===
    yloc = []
    for m in range(8):
        pt = ps[m]
        for k in range(4):
            nc.tensor.matmul(
                pt[:],
                h0t[k][:, m * 128:(m + 1) * 128],
                w1_t[k][:],
                start=(k == 0),
                stop=(k == 3),
            )
        yh = ypool.tile([128, H], F16, name=f"yh_{m}")
        nc.vector.tensor_copy(yh[:], pt[:])
        nc.sync.dma_start(cc_in[m * 128:(m + 1) * 128, :], yh[:])
        yloc.append(yh)

    # =========== AllGather Y1 (fp16) across the 8 cores ==================
    nc.gpsimd.collective_compute(
        "AllGather",
        mybir.AluOpType.bypass,
        replica_groups=[list(range(NCORES))],
        ins=[cc_in.opt()],
        outs=[cc_out.opt()],
    )

    # =========== L1 local part: own-block columns from SBUF-resident Y ===
    # A16 (streamed below) has this core's own rows zeroed on host; the own
    # contribution adj[:, own] @ Y_own runs here, overlapping the AllGather.
    for k2 in range(KB):
        ao = l1s.tile([128, BLK], F16, name="ao")
        nc.sync.dma_start(ao[:], A16own[k2 * 128:(k2 + 1) * 128, :])
        for m in range(4):
            for n in range(2):
                nc.tensor.matmul(
                    ps[m * 2 + n][:],
                    yloc[k2][:, m * 128:(m + 1) * 128],
                    ao[:, n * 512:(n + 1) * 512],
                    start=(k2 == 0),
                    stop=False,
                )

    # expert weights: needed last, emit DMA late
    wcat_t = []
    for k in range(8):
        t = const.tile([128, NWP], F32R, name=f"wcat_{k}")
        nc.sync.dma_start(t[:], Wcat[k * 128:(k + 1) * 128, :])
        wcat_t.append(t)
    iota_t = const.tile([128, E], F32, name="iota7")
    nc.sync.dma_start(iota_t[:], iota7[:, :])

    # =========== L1 streamed: all 64 k-tiles (own rows are zeros) ========
    for k in range(KT):
        yk = l1s.tile([128, H], F16, name="yk")
        ah = l1s.tile([128, BLK], F16, name="ah1")
        r = slice(k * 128, (k + 1) * 128)
        nc.sync.dma_start(yk[:], cc_out[r, :])
        nc.sync.dma_start(ah[:], A16[r, :])
        for m in range(4):
            for n in range(2):
                nc.tensor.matmul(
                    ps[m * 2 + n][:],
                    yk[:, m * 128:(m + 1) * 128],
                    ah[:, n * 512:(n + 1) * 512],
                    start=False,
                    stop=(k == KT - 1),
                )

    h1t = [acts.tile([128, BLK], F32R, name=f"h1t_{m}") for m in range(4)]
    for m in range(4):
        for n in range(2):
            nc.scalar.activation(
                h1t[m][:, n * 512:(n + 1) * 512], ps[m * 2 + n][:],
                AFT.Relu, bias=b1_t[m][:], scale=1.0 / (SCALE * SCALE),
            )

    # =========== scores + all-expert heads + one-hot select ==============
    hct = h1t + h0t
    for mc in range(8):
        pt = ps[mc]
        for k in range(8):
            nc.tensor.matmul(
                pt[:, 0:NWP],
                hct[k][:, mc * 128:(mc + 1) * 128],
                wcat_t[k][:],
                start=(k == 0),
                stop=(k == 7),
            )
        sc = pt[:, 0:E]
        oa = pt[:, E:NW]
        rmax = small.tile([128, 1], F32, name="rmax")
        nc.vector.tensor_reduce(rmax[:], sc, axis=mybir.AxisListType.X, op=ALU.max)
        val = small.tile([128, E], F32, name="val")
        nc.vector.tensor_scalar(val[:], sc, rmax[:], 1024.0, ALU.is_lt, ALU.mult)
        nc.vector.tensor_tensor(val[:], val[:], iota_t[:], op=ALU.add)
        idxf = small.tile([128, 1], F32, name="idxf")
        nc.vector.tensor_reduce(idxf[:], val[:], axis=mybir.AxisListType.X, op=ALU.min)
        onehot = small.tile([128, E], F32, name="onehot")
        nc.vector.tensor_scalar(onehot[:], val[:], idxf[:], None, ALU.is_equal)
        masked = small.tile([128, E, C], F32, name="masked")
        oa_v = oa.rearrange("p (e c) -> p e c", e=E)
        oh_v = onehot[:, :, None].broadcast_to((128, E, C))
        nc.vector.tensor_tensor(masked[:], oa_v, oh_v, op=ALU.mult)
        out_m = small.tile([128, C], F32, name="out_m")
        mv = masked[:].rearrange("p e c -> p c e")
        nc.vector.tensor_reduce(out_m[:], mv, axis=mybir.AxisListType.X, op=ALU.add)
        nc.sync.dma_start(out[mc * 128:(mc + 1) * 128, :], out_m[:])


def _build_nc():
    nc = bacc.Bacc("TRN2", target_bir_lowering=False, debug=False,
                   num_devices=NCORES)
    aps = {}
    def inp(name, shape, dt):
        aps[name] = nc.dram_tensor(name, shape, dt, kind="ExternalInput").ap()
    inp("A32", [N, BLK], F32R)
    inp("A16", [N, BLK], F16)
    inp("A16own", [BLK, BLK], F16)
    inp("Fr", [N, IN], F32R)
    inp("W0r", [IN, H], F32R)
    inp("W1r", [H, H], F32R)
    inp("b0", [H, 1], F32)
    inp("b1", [H, 1], F32)
    inp("Wcat", [2 * H, NWP], F32R)
    inp("iota7", [128, E], F32)
    aps["out"] = nc.dram_tensor("out", [BLK, C], F32, kind="ExternalOutput").ap()
    aps["cc_in"] = nc.dram_tensor("cc_in", [BLK, H], F16).ap()
    aps["cc_out"] = nc.dram_tensor("cc_out", [N, H], F16,
                                   addr_space="Shared").ap()
    from contextlib import ExitStack
    with tile.TileContext(nc) as tc, ExitStack() as ctx:
        _kernel_body(ctx, tc, aps)
    nc.compile()
    return nc


def kernel(feature, adj, W0, b0, W1, b1, Wp, Wpp):
    global LAST_RESULTS, _CACHED_NC
    feature = np.ascontiguousarray(np.asarray(feature, dtype=np.float32))
    adj = np.asarray(adj, dtype=np.float32)
    W0 = np.asarray(W0, dtype=np.float32)
    b0 = np.asarray(b0, dtype=np.float32)
    W1 = np.asarray(W1, dtype=np.float32)
    b1 = np.asarray(b1, dtype=np.float32)
    Wp = np.asarray(Wp, dtype=np.float32)
    Wpp = np.asarray(Wpp, dtype=np.float32)

    if _CACHED_NC is None:
        _CACHED_NC = _build_nc()
    nc = _CACHED_NC

    Wcat = np.concatenate(
        [Wp, Wpp.transpose(1, 0, 2).reshape(2 * H, E * C),
         np.zeros((2 * H, NWP - NW), np.float32)], axis=1)
    Wcat = np.ascontiguousarray(Wcat)
    iota7 = np.tile(np.arange(E, dtype=np.float32), (128, 1))
    shared = {
        "Fr": feature,
        "W0r": np.ascontiguousarray(W0),
        "W1r": np.ascontiguousarray(W1 * SCALE),
        "b0": b0.reshape(H, 1), "b1": b1.reshape(H, 1),
        "Wcat": Wcat, "iota7": iota7,
    }
    in_maps = []
    for c in range(NCORES):
        blk = np.ascontiguousarray(adj[c * BLK:(c + 1) * BLK, :].T)
        a16 = (blk * SCALE).astype(np.float16)
        m = dict(shared)
        m["A32"] = blk
        m["A16own"] = np.ascontiguousarray(a16[c * BLK:(c + 1) * BLK, :])
        a16 = a16.copy()
        a16[c * BLK:(c + 1) * BLK, :] = 0
        m["A16"] = np.ascontiguousarray(a16)
        in_maps.append(m)

    trace = os.environ.get("BASS_KERNEL_TRACE", "0") == "1"
    res = run_bass_kernel_spmd(nc, in_maps, list(range(NCORES)), trace=trace)
    LAST_RESULTS = res
    out = np.concatenate([res.results[c]["out"] for c in range(NCORES)], axis=0)
    return out


# revision 11
# speedup vs baseline: 1.3097x; 1.0189x over previous
"""GPPT (GCN + prompt MoE routing) Trainium2 kernel, 8-core SPMD.

Row-shards the N=8192 nodes across 8 NeuronCores (1024 rows each).
v4: v2 structure (single AllGather) + late const loads (fast start) +
single-descriptor adj DMAs + local-block L1 overlap: each core's
streamed A16 has its own block's rows zeroed on host, and the local
contribution adj[:, own] @ Y_own runs from SBUF-resident Y tiles right
after the AllGather is issued, so the PE has work during the collective.

  L0:    TT = feature^T @ adjT_blk            (single-pass fp32r)
  h0^T   = relu(W0^T @ TT + b0)               (fp32r)
  Y1s    = h0_blk @ (W1*8192)                 (fp32r) -> fp16
  AllGather(Y1 fp16, 1MB/rank)                local L1 part overlaps
  L1:    h1^T = relu((Y1s^T @ adjT16) * 2^-26 + b1)   (single-pass fp16)
  scores/experts: hc @ [Wp | WppT | pad]      (fp32r, N=256)

Precision: fp32r matmul rounds both operands to a 12-bit significand
(round-to-nearest; decoded exactly via K=1 outer-product probes and
validated against hardware to 4 digits). Host simulation of this exact
scheme on the real inputs gives 0 routing flips and rel err ~2.6e-4,
with a 1.35e-7 worst-row score margin. The L1 adjacency pass tolerates
a single fp16 pass because h1 is mean-dominated (adj >= 0, Y columns
have nonzero means), shrinking the relative impact of rounding noise.
h0/h1 must be stored at >= fp32r precision: fp16 stores flip 1-2 rows.
"""

import os
import numpy as np

import concourse.bass as bass
import concourse.mybir as mybir
import concourse.tile as tile
from concourse import bacc
from concourse.bass_utils import run_bass_kernel_spmd

N = 8192
IN = 512
H = 512
C = 32
E = 7
NCORES = 8
BLK = N // NCORES          # 1024 nodes per core
KT = N // 128              # 64 contraction k-tiles over nodes
KB = BLK // 128            # 8 k-tiles within the local block
SCALE = 8192.0             # L1 fp16 pre-scale (exact power of two)
NW = E + E * C             # 231 useful expert columns
NWP = 256                  # padded to 256 so fp32r runs 1 cycle/row

F32 = mybir.dt.float32
F32R = mybir.dt.float32r
F16 = mybir.dt.float16

LAST_RESULTS = None
_CACHED_NC = None


def _kernel_body(ctx, tc, aps):
    nc = tc.nc
    AFT = mybir.ActivationFunctionType
    ALU = mybir.AluOpType

    A32, A16, A16own = aps["A32"], aps["A16"], aps["A16own"]
    Fr = aps["Fr"]
    W0r, W1r = aps["W0r"], aps["W1r"]
    b0, b1 = aps["b0"], aps["b1"]
    Wcat = aps["Wcat"]          # [2H, 256] = [Wp | WppT | 0pad]
    iota7 = aps["iota7"]        # [128, 7] fp32 0..6 per row
    out = aps["out"]
    cc_in = aps["cc_in"]
    cc_out = [aps["cc_out0"], aps["cc_out1"]]

    const = ctx.enter_context(tc.tile_pool(name="const", bufs=1))
    acts = ctx.enter_context(tc.tile_pool(name="acts", bufs=1))
    stream = ctx.enter_context(tc.tile_pool(name="stream", bufs=6))
    l1s = ctx.enter_context(tc.tile_pool(name="l1s", bufs=4))
    ypool = ctx.enter_context(tc.tile_pool(name="ypool", bufs=1))
    small = ctx.enter_context(tc.tile_pool(name="small", bufs=4))
    psum = ctx.enter_context(tc.tile_pool(name="psum", bufs=1, space="PSUM"))

    ps = [psum.tile([128, 512], F32, name=f"bank{i}") for i in range(8)]

    # =========== L0: TT[m,n] = sum_k F[k][:,m].T @ A[k][:,n] (fp32r) =====
    # const loads are emitted AFTER the streaming loop so the first k-tiles
    # hit the DMA queues immediately at kernel start.
    for k in range(KT):
        ft = stream.tile([128, IN], F32R, name="ft")
        at = stream.tile([128, BLK], F32R, name="at")
        r = slice(k * 128, (k + 1) * 128)
        nc.sync.dma_start(ft[:], Fr[r, :])
        nc.sync.dma_start(at[:], A32[r, :])
        for m in range(4):
            for n in range(2):
                nc.tensor.matmul(
                    ps[m * 2 + n][:],
                    ft[:, m * 128:(m + 1) * 128],
                    at[:, n * 512:(n + 1) * 512],
                    start=(k == 0),
                    stop=(k == KT - 1),
                )

    # ---- weights needed from the h0 phase onward ----
    w0_t = []
    w1_t = []
    for k in range(4):
        t = const.tile([128, H], F32R, name=f"w0_{k}")
        nc.sync.dma_start(t[:], W0r[k * 128:(k + 1) * 128, :])
        w0_t.append(t)
        t = const.tile([128, H], F32R, name=f"w1_{k}")
        nc.sync.dma_start(t[:], W1r[k * 128:(k + 1) * 128, :])
        w1_t.append(t)
    b0_t = []
    b1_t = []
    for m in range(4):
        t = const.tile([128, 1], F32, name=f"b0_{m}")
        nc.sync.dma_start(t[:], b0[m * 128:(m + 1) * 128, :])
        b0_t.append(t)
        t = const.tile([128, 1], F32, name=f"b1_{m}")
        nc.sync.dma_start(t[:], b1[m * 128:(m + 1) * 128, :])
        b1_t.append(t)

    # copy TT out of PSUM
    tt = []
    for m in range(4):
        t = acts.tile([128, BLK], F32R, name=f"tt_{m}")
        for n in range(2):
            nc.vector.tensor_copy(t[:, n * 512:(n + 1) * 512], ps[m * 2 + n][:])
        tt.append(t)

    # =========== h0T[m,n] = relu(sum_k W0[k][:,m].T @ TT[k][:,n] + b0) ===
    h0t = [acts.tile([128, BLK], F32R, name=f"h0t_{m}") for m in range(4)]
    for m in range(4):
        for n in range(2):
            pt = ps[m * 2 + n]
            for k in range(4):
                nc.tensor.matmul(
                    pt[:],
                    w0_t[k][:, m * 128:(m + 1) * 128],
                    tt[k][:, n * 512:(n + 1) * 512],
                    start=(k == 0),
                    stop=(k == 3),
                )
            nc.scalar.activation(
                h0t[m][:, n * 512:(n + 1) * 512], pt[:],
                AFT.Relu, bias=b0_t[m][:], scale=1.0,
            )

    # =========== Y1s[m] = sum_k h0t[k][:,m].T @ W1r[k]  -> fp16 chunks ===
    yloc = []
    for m in range(8):
        pt = ps[m]
        for k in range(4):
            nc.tensor.matmul(
                pt[:],
                h0t[k][:, m * 128:(m + 1) * 128],
                w1_t[k][:],
                start=(k == 0),
                stop=(k == 3),
            )
        yh = ypool.tile([128, H], F16, name=f"yh_{m}")
        nc.vector.tensor_copy(yh[:], pt[:])
        nc.sync.dma_start(cc_in[m * 128:(m + 1) * 128, :], yh[:])
        yloc.append(yh)

    # ===== AllGather Y1 (fp16) in two halves; L1 consumes half 0 first ===
    for h in range(2):
        nc.gpsimd.collective_compute(
            "AllGather",
            mybir.AluOpType.bypass,
            replica_groups=[list(range(NCORES))],
            ins=[cc_in[h * (BLK // 2):(h + 1) * (BLK // 2), :].opt()],
            outs=[cc_out[h].opt()],
        )

    # =========== L1 local part: own-block columns from SBUF-resident Y ===
    # A16 (streamed below) has this core's own rows zeroed on host; the own
    # contribution adj[:, own] @ Y_own runs here, overlapping the AllGather.
    for k2 in range(KB):
        ao = l1s.tile([128, BLK], F16, name="ao")
        nc.sync.dma_start(ao[:], A16own[k2 * 128:(k2 + 1) * 128, :])
        for m in range(4):
            for n in range(2):
                nc.tensor.matmul(
                    ps[m * 2 + n][:],
                    yloc[k2][:, m * 128:(m + 1) * 128],
                    ao[:, n * 512:(n + 1) * 512],
                    start=(k2 == 0),
                    stop=False,
                )

    # expert weights: needed last, emit DMA late
    wcat_t = []
    for k in range(8):
        t = const.tile([128, NWP], F32R, name=f"wcat_{k}")
        nc.sync.dma_start(t[:], Wcat[k * 128:(k + 1) * 128, :])
        wcat_t.append(t)
    iota_t = const.tile([128, E], F32, name="iota7")
    nc.sync.dma_start(iota_t[:], iota7[:, :])

    # =========== L1 streamed: all 64 k-tiles (own rows are zeros) ========
    # reordered: tiles gathered by AG half 0 first, then half 1
    korder = [k for k in range(KT) if k % 8 < 4] + \
             [k for k in range(KT) if k % 8 >= 4]
    for k in korder:
        g = k * 128
        rank, w = g // BLK, g % BLK
        src = cc_out[0] if w < BLK // 2 else cc_out[1]
        row = rank * (BLK // 2) + (w % (BLK // 2))
        yk = l1s.tile([128, H], F16, name="yk")
        ah = l1s.tile([128, BLK], F16, name="ah1")
        r = slice(k * 128, (k + 1) * 128)
        nc.sync.dma_start(yk[:], src[row:row + 128, :])
        nc.sync.dma_start(ah[:], A16[r, :])
        for m in range(4):
            for n in range(2):
                nc.tensor.matmul(
                    ps[m * 2 + n][:],
                    yk[:, m * 128:(m + 1) * 128],
                    ah[:, n * 512:(n + 1) * 512],
                    start=False,
                    stop=(k == korder[-1]),
                )

    h1t = [acts.tile([128, BLK], F32R, name=f"h1t_{m}") for m in range(4)]
    for m in range(4):
        for n in range(2):
            nc.scalar.activation(
                h1t[m][:, n * 512:(n + 1) * 512], ps[m * 2 + n][:],
                AFT.Relu, bias=b1_t[m][:], scale=1.0 / (SCALE * SCALE),
            )

    # =========== scores + all-expert heads + one-hot select ==============
    hct = h1t + h0t
    for mc in range(8):
        pt = ps[mc]
        for k in range(8):
            nc.tensor.matmul(
                pt[:, 0:NWP],
                hct[k][:, mc * 128:(mc + 1) * 128],
                wcat_t[k][:],
                start=(k == 0),
                stop=(k == 7),
            )
        sc = pt[:, 0:E]
        oa = pt[:, E:NW]
        rmax = small.tile([128, 1], F32, name="rmax")
        nc.vector.tensor_reduce(rmax[:], sc, axis=mybir.AxisListType.X, op=ALU.max)
        val = small.tile([128, E], F32, name="val")
        nc.vector.tensor_scalar(val[:], sc, rmax[:], 1024.0, ALU.is_lt, ALU.mult)
        nc.vector.tensor_tensor(val[:], val[:], iota_t[:], op=ALU.add)
        idxf = small.tile([128, 1], F32, name="idxf")
        nc.vector.tensor_reduce(idxf[:], val[:], axis=mybir.AxisListType.X, op=ALU.min)
        onehot = small.tile([128, E], F32, name="onehot")
        nc.vector.tensor_scalar(onehot[:], val[:], idxf[:], None, ALU.is_equal)
        masked = small.tile([128, E, C], F32, name="masked")
        oa_v = oa.rearrange("p (e c) -> p e c", e=E)
        oh_v = onehot[:, :, None].broadcast_to((128, E, C))
        nc.vector.tensor_tensor(masked[:], oa_v, oh_v, op=ALU.mult)
        out_m = small.tile([128, C], F32, name="out_m")
        mv = masked[:].rearrange("p e c -> p c e")
        nc.vector.tensor_reduce(out_m[:], mv, axis=mybir.AxisListType.X, op=ALU.add)
        nc.sync.dma_start(out[mc * 128:(mc + 1) * 128, :], out_m[:])


def _build_nc():
    nc = bacc.Bacc("TRN2", target_bir_lowering=False, debug=False,
                   num_devices=NCORES)
    aps = {}
    def inp(name, shape, dt):
        aps[name] = nc.dram_tensor(name, shape, dt, kind="ExternalInput").ap()
    inp("A32", [N, BLK], F32R)
    inp("A16", [N, BLK], F16)
    inp("A16own", [BLK, BLK], F16)
    inp("Fr", [N, IN], F32R)
    inp("W0r", [IN, H], F32R)
    inp("W1r", [H, H], F32R)
    inp("b0", [H, 1], F32)
    inp("b1", [H, 1], F32)
    inp("Wcat", [2 * H, NWP], F32R)
    inp("iota7", [128, E], F32)
    aps["out"] = nc.dram_tensor("out", [BLK, C], F32, kind="ExternalOutput").ap()
    aps["cc_in"] = nc.dram_tensor("cc_in", [BLK, H], F16).ap()
    aps["cc_out0"] = nc.dram_tensor("cc_out0", [N // 2, H], F16,
                                    addr_space="Shared").ap()
    aps["cc_out1"] = nc.dram_tensor("cc_out1", [N // 2, H], F16,
                                    addr_space="Shared").ap()
    from contextlib import ExitStack
    with tile.TileContext(nc) as tc, ExitStack() as ctx:
        _kernel_body(ctx, tc, aps)
    nc.compile()
    return nc


def kernel(feature, adj, W0, b0, W1, b1, Wp, Wpp):
    global LAST_RESULTS, _CACHED_NC
    feature = np.ascontiguousarray(np.asarray(feature, dtype=np.float32))
    adj = np.asarray(adj, dtype=np.float32)
    W0 = np.asarray(W0, dtype=np.float32)
    b0 = np.asarray(b0, dtype=np.float32)
    W1 = np.asarray(W1, dtype=np.float32)
    b1 = np.asarray(b1, dtype=np.float32)
    Wp = np.asarray(Wp, dtype=np.float32)
    Wpp = np.asarray(Wpp, dtype=np.float32)

    if _CACHED_NC is None:
        _CACHED_NC = _build_nc()
    nc = _CACHED_NC

    Wcat = np.concatenate(
        [Wp, Wpp.transpose(1, 0, 2).reshape(2 * H, E * C),
         np.zeros((2 * H, NWP - NW), np.float32)], axis=1)
    Wcat = np.ascontiguousarray(Wcat)
    iota7 = np.tile(np.arange(E, dtype=np.float32), (128, 1))
    shared = {
        "Fr": feature,
        "W0r": np.ascontiguousarray(W0),
        "W1r": np.ascontiguousarray(W1 * SCALE),
        "b0": b0.reshape(H, 1), "b1": b1.reshape(H, 1),
        "Wcat": Wcat, "iota7": iota7,
    }
    in_maps = []
    for c in range(NCORES):
        blk = np.ascontiguousarray(adj[c * BLK:(c + 1) * BLK, :].T)
        a16 = (blk * SCALE).astype(np.float16)
        m = dict(shared)
        m["A32"] = blk
        m["A16own"] = np.ascontiguousarray(a16[c * BLK:(c + 1) * BLK, :])
        a16 = a16.copy()
        a16[c * BLK:(c + 1) * BLK, :] = 0
        m["A16"] = np.ascontiguousarray(a16)
        in_maps.append(m)

    trace = os.environ.get("BASS_KERNEL_TRACE", "0") == "1"
    res = run_bass_kernel_spmd(nc, in_maps, list(range(NCORES)), trace=trace)
    LAST_RESULTS = res
    out = np.concatenate([res.results[c]["out"] for c in range(NCORES)], axis=0)
    return out


# revision 12
# speedup vs baseline: 1.3601x; 1.0385x over previous
"""GPPT (GCN + prompt MoE routing) Trainium2 kernel, 8-core SPMD.

Row-shards the N=8192 nodes across 8 NeuronCores (1024 rows each).
v4: v2 structure (single AllGather) + late const loads (fast start) +
single-descriptor adj DMAs + local-block L1 overlap: each core's
streamed A16 has its own block's rows zeroed on host, and the local
contribution adj[:, own] @ Y_own runs from SBUF-resident Y tiles right
after the AllGather is issued, so the PE has work during the collective.

  L0:    TT = feature^T @ adjT_blk            (single-pass fp32r)
  h0^T   = relu(W0^T @ TT + b0)               (fp32r)
  Y1s    = h0_blk @ (W1*8192)                 (fp32r) -> fp16
  AllGather(Y1 fp16, 1MB/rank)                local L1 part overlaps
  L1:    h1^T = relu((Y1s^T @ adjT16) * 2^-26 + b1)   (single-pass fp16)
  scores/experts: hc @ [Wp | WppT | pad]      (fp32r, N=256)

Precision: fp32r matmul rounds both operands to a 12-bit significand
(round-to-nearest; decoded exactly via K=1 outer-product probes and
validated against hardware to 4 digits). Host simulation of this exact
scheme on the real inputs gives 0 routing flips and rel err ~2.6e-4,
with a 1.35e-7 worst-row score margin. The L1 adjacency pass tolerates
a single fp16 pass because h1 is mean-dominated (adj >= 0, Y columns
have nonzero means), shrinking the relative impact of rounding noise.
h0/h1 must be stored at >= fp32r precision: fp16 stores flip 1-2 rows.
"""

import os
import numpy as np

import concourse.bass as bass
import concourse.mybir as mybir
import concourse.tile as tile
from concourse import bacc
from concourse.bass_utils import run_bass_kernel_spmd

N = 8192
IN = 512
H = 512
C = 32
E = 7
NCORES = 8
BLK = N // NCORES          # 1024 nodes per core
KT = N // 128              # 64 contraction k-tiles over nodes
KB = BLK // 128            # 8 k-tiles within the local block
SCALE = 8192.0             # L1 fp16 pre-scale (exact power of two)
NW = E + E * C             # 231 useful expert columns
NWP = 256                  # padded to 256 so fp32r runs 1 cycle/row

F32 = mybir.dt.float32
F32R = mybir.dt.float32r
F16 = mybir.dt.float16

LAST_RESULTS = None
_CACHED_NC = None


def _kernel_body(ctx, tc, aps):
    nc = tc.nc
    AFT = mybir.ActivationFunctionType
    ALU = mybir.AluOpType

    A32, A16, A16own = aps["A32"], aps["A16"], aps["A16own"]
    Fr = aps["Fr"]
    W0r, W1r = aps["W0r"], aps["W1r"]
    b0, b1 = aps["b0"], aps["b1"]
    Wcat = aps["Wcat"]          # [2H, 256] = [Wp | WppT | 0pad]
    iota7 = aps["iota7"]        # [128, 7] fp32 0..6 per row
    out = aps["out"]
    cc_in = aps["cc_in"]
    cc_out = [aps[f"cc_out{q}"] for q in range(4)]

    const = ctx.enter_context(tc.tile_pool(name="const", bufs=1))
    acts = ctx.enter_context(tc.tile_pool(name="acts", bufs=1))
    stream = ctx.enter_context(tc.tile_pool(name="stream", bufs=6))
    l1s = ctx.enter_context(tc.tile_pool(name="l1s", bufs=4))
    ypool = ctx.enter_context(tc.tile_pool(name="ypool", bufs=1))
    small = ctx.enter_context(tc.tile_pool(name="small", bufs=4))
    psum = ctx.enter_context(tc.tile_pool(name="psum", bufs=1, space="PSUM"))

    ps = [psum.tile([128, 512], F32, name=f"bank{i}") for i in range(8)]

    # =========== L0: TT[m,n] = sum_k F[k][:,m].T @ A[k][:,n] (fp32r) =====
    # const loads are emitted AFTER the streaming loop so the first k-tiles
    # hit the DMA queues immediately at kernel start.
    for k in range(KT):
        ft = stream.tile([128, IN], F32R, name="ft")
        at = stream.tile([128, BLK], F32R, name="at")
        r = slice(k * 128, (k + 1) * 128)
        nc.sync.dma_start(ft[:], Fr[r, :])
        nc.sync.dma_start(at[:], A32[r, :])
        for m in range(4):
            for n in range(2):
                nc.tensor.matmul(
                    ps[m * 2 + n][:],
                    ft[:, m * 128:(m + 1) * 128],
                    at[:, n * 512:(n + 1) * 512],
                    start=(k == 0),
                    stop=(k == KT - 1),
                )

    # ---- weights needed from the h0 phase onward ----
    w0_t = []
    w1_t = []
    for k in range(4):
        t = const.tile([128, H], F32R, name=f"w0_{k}")
        nc.sync.dma_start(t[:], W0r[k * 128:(k + 1) * 128, :])
        w0_t.append(t)
        t = const.tile([128, H], F32R, name=f"w1_{k}")
        nc.sync.dma_start(t[:], W1r[k * 128:(k + 1) * 128, :])
        w1_t.append(t)
    b0_t = []
    b1_t = []
    for m in range(4):
        t = const.tile([128, 1], F32, name=f"b0_{m}")
        nc.sync.dma_start(t[:], b0[m * 128:(m + 1) * 128, :])
        b0_t.append(t)
        t = const.tile([128, 1], F32, name=f"b1_{m}")
        nc.sync.dma_start(t[:], b1[m * 128:(m + 1) * 128, :])
        b1_t.append(t)

    # copy TT out of PSUM
    tt = []
    for m in range(4):
        t = acts.tile([128, BLK], F32R, name=f"tt_{m}")
        for n in range(2):
            nc.vector.tensor_copy(t[:, n * 512:(n + 1) * 512], ps[m * 2 + n][:])
        tt.append(t)

    # =========== h0T[m,n] = relu(sum_k W0[k][:,m].T @ TT[k][:,n] + b0) ===
    h0t = [acts.tile([128, BLK], F32R, name=f"h0t_{m}") for m in range(4)]
    for n in range(2):
        for m in range(4):
            pt = ps[m * 2 + n]
            for k in range(4):
                nc.tensor.matmul(
                    pt[:],
                    w0_t[k][:, m * 128:(m + 1) * 128],
                    tt[k][:, n * 512:(n + 1) * 512],
                    start=(k == 0),
                    stop=(k == 3),
                )
            nc.scalar.activation(
                h0t[m][:, n * 512:(n + 1) * 512], pt[:],
                AFT.Relu, bias=b0_t[m][:], scale=1.0,
            )

    # =========== Y1s[m] = sum_k h0t[k][:,m].T @ W1r[k]  -> fp16 chunks ===
    yloc = []
    for m in range(8):
        pt = ps[m]
        for k in range(4):
            nc.tensor.matmul(
                pt[:],
                h0t[k][:, m * 128:(m + 1) * 128],
                w1_t[k][:],
                start=(k == 0),
                stop=(k == 3),
            )
        yh = ypool.tile([128, H], F16, name=f"yh_{m}")
        nc.vector.tensor_copy(yh[:], pt[:])
        nc.sync.dma_start(cc_in[m * 128:(m + 1) * 128, :], yh[:])
        yloc.append(yh)

    # == AllGather Y1 (fp16) in four quarters; L1 consumes them in order ==
    QB = BLK // 4
    for q in range(4):
        nc.gpsimd.collective_compute(
            "AllGather",
            mybir.AluOpType.bypass,
            replica_groups=[list(range(NCORES))],
            ins=[cc_in[q * QB:(q + 1) * QB, :].opt()],
            outs=[cc_out[q].opt()],
        )

    # =========== L1 local part: own-block columns from SBUF-resident Y ===
    # A16 (streamed below) has this core's own rows zeroed on host; the own
    # contribution adj[:, own] @ Y_own runs here, overlapping the AllGather.
    for k2 in range(KB):
        ao = l1s.tile([128, BLK], F16, name="ao")
        nc.sync.dma_start(ao[:], A16own[k2 * 128:(k2 + 1) * 128, :])
        for m in range(4):
            for n in range(2):
                nc.tensor.matmul(
                    ps[m * 2 + n][:],
                    yloc[k2][:, m * 128:(m + 1) * 128],
                    ao[:, n * 512:(n + 1) * 512],
                    start=(k2 == 0),
                    stop=False,
                )

    # expert weights: needed last, emit DMA late
    wcat_t = []
    for k in range(8):
        t = const.tile([128, NWP], F32R, name=f"wcat_{k}")
        nc.sync.dma_start(t[:], Wcat[k * 128:(k + 1) * 128, :])
        wcat_t.append(t)
    iota_t = const.tile([128, E], F32, name="iota7")
    nc.sync.dma_start(iota_t[:], iota7[:, :])

    # =========== L1 streamed: all 64 k-tiles (own rows are zeros) ========
    # reordered: tiles gathered by AG quarter q first, in quarter order
    korder = [k for q in range(4) for k in range(KT) if (k % 8) // 2 == q]
    for k in korder:
        g = k * 128
        rank, w = g // BLK, g % BLK
        q = w // (BLK // 4)
        src = cc_out[q]
        row = rank * (BLK // 4) + (w % (BLK // 4))
        yk = l1s.tile([128, H], F16, name="yk")
        ah = l1s.tile([128, BLK], F16, name="ah1")
        r = slice(k * 128, (k + 1) * 128)
        nc.sync.dma_start(yk[:], src[row:row + 128, :])
        nc.sync.dma_start(ah[:], A16[r, :])
        for m in range(4):
            for n in range(2):
                nc.tensor.matmul(
                    ps[m * 2 + n][:],
                    yk[:, m * 128:(m + 1) * 128],
                    ah[:, n * 512:(n + 1) * 512],
                    start=False,
                    stop=(k == korder[-1]),
                )

    h1t = [acts.tile([128, BLK], F32R, name=f"h1t_{m}") for m in range(4)]
    for m in range(4):
        for n in range(2):
            nc.scalar.activation(
                h1t[m][:, n * 512:(n + 1) * 512], ps[m * 2 + n][:],
                AFT.Relu, bias=b1_t[m][:], scale=1.0 / (SCALE * SCALE),
            )

    # =========== scores + all-expert heads + one-hot select ==============
    hct = h1t + h0t
    for mc in range(8):
        pt = ps[mc]
        for k in range(8):
            nc.tensor.matmul(
                pt[:, 0:NWP],
                hct[k][:, mc * 128:(mc + 1) * 128],
                wcat_t[k][:],
                start=(k == 0),
                stop=(k == 7),
            )
        sc = pt[:, 0:E]
        oa = pt[:, E:NW]
        rmax = small.tile([128, 1], F32, name="rmax")
        nc.vector.tensor_reduce(rmax[:], sc, axis=mybir.AxisListType.X, op=ALU.max)
        val = small.tile([128, E], F32, name="val")
        nc.vector.tensor_scalar(val[:], sc, rmax[:], 1024.0, ALU.is_lt, ALU.mult)
        nc.vector.tensor_tensor(val[:], val[:], iota_t[:], op=ALU.add)
        idxf = small.tile([128, 1], F32, name="idxf")
        nc.vector.tensor_reduce(idxf[:], val[:], axis=mybir.AxisListType.X, op=ALU.min)
        onehot = small.tile([128, E], F32, name="onehot")
        nc.vector.tensor_scalar(onehot[:], val[:], idxf[:], None, ALU.is_equal)
        masked = small.tile([128, E, C], F32, name="masked")
        oa_v = oa.rearrange("p (e c) -> p e c", e=E)
        oh_v = onehot[:, :, None].broadcast_to((128, E, C))
        nc.vector.tensor_tensor(masked[:], oa_v, oh_v, op=ALU.mult)
        out_m = small.tile([128, C], F32, name="out_m")
        mv = masked[:].rearrange("p e c -> p c e")
        nc.vector.tensor_reduce(out_m[:], mv, axis=mybir.AxisListType.X, op=ALU.add)
        nc.sync.dma_start(out[mc * 128:(mc + 1) * 128, :], out_m[:])


def _build_nc():
    nc = bacc.Bacc("TRN2", target_bir_lowering=False, debug=False,
                   num_devices=NCORES)
    aps = {}
    def inp(name, shape, dt):
        aps[name] = nc.dram_tensor(name, shape, dt, kind="ExternalInput").ap()
    inp("A32", [N, BLK], F32R)
    inp("A16", [N, BLK], F16)
    inp("A16own", [BLK, BLK], F16)
    inp("Fr", [N, IN], F32R)
    inp("W0r", [IN, H], F32R)
    inp("W1r", [H, H], F32R)
    inp("b0", [H, 1], F32)
    inp("b1", [H, 1], F32)
    inp("Wcat", [2 * H, NWP], F32R)
    inp("iota7", [128, E], F32)
    aps["out"] = nc.dram_tensor("out", [BLK, C], F32, kind="ExternalOutput").ap()
    aps["cc_in"] = nc.dram_tensor("cc_in", [BLK, H], F16).ap()
    for q in range(4):
        aps[f"cc_out{q}"] = nc.dram_tensor(f"cc_out{q}", [N // 4, H], F16,
                                           addr_space="Shared").ap()
    from contextlib import ExitStack
    with tile.TileContext(nc) as tc, ExitStack() as ctx:
        _kernel_body(ctx, tc, aps)
    nc.compile()
    return nc


def kernel(feature, adj, W0, b0, W1, b1, Wp, Wpp):
    global LAST_RESULTS, _CACHED_NC
    feature = np.ascontiguousarray(np.asarray(feature, dtype=np.float32))
    adj = np.asarray(adj, dtype=np.float32)
    W0 = np.asarray(W0, dtype=np.float32)
    b0 = np.asarray(b0, dtype=np.float32)
    W1 = np.asarray(W1, dtype=np.float32)
    b1 = np.asarray(b1, dtype=np.float32)
    Wp = np.asarray(Wp, dtype=np.float32)
    Wpp = np.asarray(Wpp, dtype=np.float32)

    if _CACHED_NC is None:
        _CACHED_NC = _build_nc()
    nc = _CACHED_NC

    Wcat = np.concatenate(
        [Wp, Wpp.transpose(1, 0, 2).reshape(2 * H, E * C),
         np.zeros((2 * H, NWP - NW), np.float32)], axis=1)
    Wcat = np.ascontiguousarray(Wcat)
    iota7 = np.tile(np.arange(E, dtype=np.float32), (128, 1))
    shared = {
        "Fr": feature,
        "W0r": np.ascontiguousarray(W0),
        "W1r": np.ascontiguousarray(W1 * SCALE),
        "b0": b0.reshape(H, 1), "b1": b1.reshape(H, 1),
        "Wcat": Wcat, "iota7": iota7,
    }
    in_maps = []
    for c in range(NCORES):
        blk = np.ascontiguousarray(adj[c * BLK:(c + 1) * BLK, :].T)
        a16 = (blk * SCALE).astype(np.float16)
        m = dict(shared)
        m["A32"] = blk
        m["A16own"] = np.ascontiguousarray(a16[c * BLK:(c + 1) * BLK, :])
        a16 = a16.copy()
        a16[c * BLK:(c + 1) * BLK, :] = 0
        m["A16"] = np.ascontiguousarray(a16)
        in_maps.append(m)

    trace = os.environ.get("BASS_KERNEL_TRACE", "0") == "1"
    res = run_bass_kernel_spmd(nc, in_maps, list(range(NCORES)), trace=trace)
    LAST_RESULTS = res
    out = np.concatenate([res.results[c]["out"] for c in range(NCORES)], axis=0)
    return out
